# revision 13
# baseline (speedup 1.0000x reference)
"""Multi-head self-attention block (LN -> QKV -> attention -> proj -> residual)
for Trainium2, data-parallel over batch across 8 NeuronCores.

Self-contained: hardcodes shapes B=8, C=384, H=W=32 (N=1024 tokens), 8 heads,
head_dim=48. Each core processes one batch element; inputs are sharded and
outputs gathered on the host.

On-chip design notes (per core, everything "feature-major" [feat, token]):
- x[b] is [C, N] in memory, which is exactly t^T for the matmuls; LayerNorm
  statistics are computed with ones-vector matmuls (sums over the partition
  dim), and normalization t = x*R + Q is folded into the QKV matmul epilogue:
  qkvT = R .* (W'^T.T @ x + muneg*colsum + invR*bias) via K=2 affine matmuls.
- ln_w/ln_b are folded into the QKV weights host-side (W' = w_qkv * ln_w,
  bias = w_qkv @ ln_b).
- Heads padded 48->64 and packed in pairs: S^T = k q^T per head pair via two
  row-tiled (tile_position) bf16 matmuls; exp via ScalarE (scale folded in);
  AV via two col-tiled bf16 matmuls with an extra ones-column in v producing
  the softmax denominator for free; normalize by mask-matmul-broadcast
  fast reciprocal at the pair level.
- proj contracts over the padded layout (host-padded w_proj kills junk rows);
  residual added from the original fp32 x.
"""

import numpy as np
import ml_dtypes

B = 8
C = 384
N = 1024
NH = 8
D = 48
PAIRS = 4
EPS = 1e-6
SCALE = float(D) ** -0.5

_CACHE = {}


def _build():
    import concourse.bacc as bacc
    import concourse.bass as bass
    import concourse.mybir as mybir
    from concourse.tile import TileContext

    f32 = mybir.dt.float32
    f32r = mybir.dt.float32r
    bf16 = mybir.dt.bfloat16
    AF = mybir.ActivationFunctionType
    MULT = mybir.AluOpType.mult
    ADD = mybir.AluOpType.add
    SUB = mybir.AluOpType.subtract

    nc = bacc.Bacc(None, target_bir_lowering=False, debug=True)

    # ---- DRAM I/O ----
    x_d = nc.dram_tensor("x", [C, N], f32, kind="ExternalInput")
    xb_d = nc.dram_tensor("xb", [C, N], bf16, kind="ExternalInput")
    wq_d = nc.dram_tensor("wq", [C, 512], bf16, kind="ExternalInput")
    wk_d = nc.dram_tensor("wk", [C, 512], bf16, kind="ExternalInput")
    wv_d = nc.dram_tensor("wv", [C, 512], bf16, kind="ExternalInput")
    wp_d = nc.dram_tensor("wp", [512, C], bf16, kind="ExternalInput")
    cbq_d = nc.dram_tensor("cbq", [2, 512], bf16, kind="ExternalInput")
    cbk_d = nc.dram_tensor("cbk", [2, 512], bf16, kind="ExternalInput")
    cbv_d = nc.dram_tensor("cbv", [2, 512], bf16, kind="ExternalInput")
    ones128_d = nc.dram_tensor("ones128", [128, 1], bf16, kind="ExternalInput")
    ones1r_d = nc.dram_tensor("ones1r", [1, 128], f32r, kind="ExternalInput")
    maskA_d = nc.dram_tensor("maskA", [1, 128], f32r, kind="ExternalInput")
    maskB_d = nc.dram_tensor("maskB", [1, 128], f32r, kind="ExternalInput")
    out_d = nc.dram_tensor("out", [C, N], f32, kind="ExternalOutput")
    dscr_d = [nc.dram_tensor(f"dscr{j}", [2, N], f32) for j in range(PAIRS)]

    with TileContext(nc) as tc:
        with (
            tc.tile_pool(name="sbw", bufs=1) as sbw,      # weights/consts
            tc.tile_pool(name="sbx", bufs=1) as sbx,      # x, xb, outputs
            tc.tile_pool(name="sbt", bufs=1) as sbt,      # qT/kT/v/attnT tiles
            tc.tile_pool(name="sbr", bufs=1) as sbr,      # small rows
            tc.tile_pool(name="sbs", bufs=2) as sbs,      # scratch
            tc.tile_pool(name="sbu", bufs=10) as sbu,      # U (exp) tiles
            tc.tile_pool(name="psA", bufs=3, space="PSUM") as psA,  # G/S/stats/P
            tc.tile_pool(name="psC", bufs=1, space="PSUM") as psC,  # sq-stats/av
        ):
            # ---- loads ----
            x_sb = [sbx.tile([128, N], f32, tag=f"x{c}", name=f"x{c}")
                    for c in range(3)]
            xb_sb = [sbx.tile([128, N], bf16, tag=f"xb{c}", name=f"xb{c}")
                     for c in range(3)]
            for c in range(3):
                nc.sync.dma_start(out=xb_sb[c], in_=xb_d[128 * c:128 * (c + 1), :])
            wq_sb = [sbw.tile([128, 512], bf16, tag=f"wq{c}", name=f"wq{c}")
                     for c in range(3)]
            wk_sb = [sbw.tile([128, 512], bf16, tag=f"wk{c}", name=f"wk{c}")
                     for c in range(3)]
            wv_sb = [sbw.tile([128, 512], bf16, tag=f"wv{c}", name=f"wv{c}")
                     for c in range(3)]
            for c in range(3):
                nc.sync.dma_start(out=wq_sb[c], in_=wq_d[128 * c:128 * (c + 1), :])
                nc.sync.dma_start(out=wk_sb[c], in_=wk_d[128 * c:128 * (c + 1), :])
                nc.sync.dma_start(out=wv_sb[c], in_=wv_d[128 * c:128 * (c + 1), :])
            wp_sb = [sbw.tile([128, C], bf16, tag=f"wp{r}", name=f"wp{r}")
                     for r in range(4)]
            for r in range(4):
                nc.sync.dma_start(out=wp_sb[r], in_=wp_d[128 * r:128 * (r + 1), :])
            cbq_sb = sbw.tile([2, 512], bf16, tag="cbq")
            cbk_sb = sbw.tile([2, 512], bf16, tag="cbk")
            cbv_sb = sbw.tile([2, 512], bf16, tag="cbv")
            nc.scalar.dma_start(out=cbq_sb, in_=cbq_d[:, :])
            nc.scalar.dma_start(out=cbk_sb, in_=cbk_d[:, :])
            nc.scalar.dma_start(out=cbv_sb, in_=cbv_d[:, :])
            ones128 = sbw.tile([128, 1], bf16, tag="ones128")
            ones1r = sbw.tile([1, 128], f32r, tag="ones1r")
            maskA = sbw.tile([1, 128], f32r, tag="maskA")
            maskB = sbw.tile([1, 128], f32r, tag="maskB")
            nc.scalar.dma_start(out=ones128, in_=ones128_d[:, :])
            nc.scalar.dma_start(out=ones1r, in_=ones1r_d[:, :])
            nc.scalar.dma_start(out=maskA, in_=maskA_d[:, :])
            nc.scalar.dma_start(out=maskB, in_=maskB_d[:, :])
            for c in range(3):
                nc.sync.dma_start(out=x_sb[c], in_=x_d[128 * c:128 * (c + 1), :])
            eps_t = sbr.tile([128, 1], f32, tag="eps")
            nc.vector.memset(eps_t, EPS)
            one1x1 = sbr.tile([1, 1], f32, tag="one1x1")
            nc.vector.memset(one1x1, 1.0)

            # ---- LayerNorm statistics ----
            # su = sum(x) [1, N] (psB slot); ssq = sum(x^2) [1, N] (psC slot,
            # free until attention starts)
            stats_su = psA.tile([128, N], f32, tag="A", name="stats_su")
            stats_sq = psC.tile([128, N], f32, tag="C", name="stats_sq")
            for half in range(2):
                hs = slice(512 * half, 512 * (half + 1))
                for c in range(3):
                    nc.tensor.matmul(
                        stats_su[0:1, hs], ones128, xb_sb[c][:, hs],
                        start=(c == 0), stop=(c == 2),
                    )
            for c in range(3):
                sq = sbs.tile([128, N], bf16, tag="sq")
                nc.vector.tensor_tensor(sq, xb_sb[c], xb_sb[c], MULT)
                for half in range(2):
                    hs = slice(512 * half, 512 * (half + 1))
                    nc.tensor.matmul(
                        stats_sq[0:1, hs], ones128, sq[:, hs],
                        start=(c == 0), stop=(c == 2),
                    )

            su = stats_su[0:1, :]
            sq_row = stats_sq[0:1, :]
            su_sb = sbr.tile([1, N], f32, tag="su_sb")
            nc.vector.tensor_copy(su_sb, su)
            z = sbr.tile([1, N], f32, tag="z")
            nc.vector.tensor_tensor(z, su_sb, su_sb, MULT)
            z2 = sbr.tile([1, N], f32, tag="z2")
            nc.vector.tensor_scalar(
                out=z2, in0=z, scalar1=1.0 / C, scalar2=None, op0=MULT
            )
            w_r = sbr.tile([1, N], f32, tag="w")
            nc.vector.tensor_tensor(w_r, sq_row, z2, SUB)
            u_r = sbr.tile([1, N], f32, tag="u")
            nc.scalar.activation(
                u_r, w_r, AF.Ln, scale=1.0 / C, bias=eps_t[0:1, :]
            )
            # R = rstd = exp(-u/2);  invR = exp(+u/2)
            R32 = sbr.tile([1, N], f32, tag="R32")
            nc.scalar.activation(R32, u_r, AF.Exp, scale=-0.5)
            Rr = sbr.tile([1, N], f32r, tag="Rr")
            nc.vector.tensor_copy(Rr, R32)
            # QV2 = [muneg; invR] bf16 rows for the affine matmuls
            qv2 = sbr.tile([2, N], bf16, tag="qv2")
            nc.vector.tensor_scalar(
                out=qv2[0:1, :], in0=su, scalar1=-1.0 / C, scalar2=None, op0=MULT
            )
            invR = sbr.tile([1, N], bf16, tag="invR")
            nc.scalar.activation(invR, u_r, AF.Exp, scale=0.5)
            nc.sync.dma_start(out=qv2[1:2, :], in_=invR)

            # R as a column per token chunk: [128, 8], col m = R[128m:128m+128]
            prcol = psA.tile([128, 8], f32, tag="A", name="prcol")
            for m in range(8):
                nc.tensor.matmul(
                    prcol[:, m:m + 1],
                    R32[0:1, 128 * m:128 * (m + 1)],
                    one1x1[0:1, 0:1],
                    start=True, stop=True,
                )
            rcol = sbr.tile([128, 8], f32, tag="rcol")
            nc.vector.tensor_copy(rcol, prcol)
            # R broadcast across partitions (K=1 matmuls, short critical path)
            prbc = psA.tile([128, N], f32, tag="A", name="prbc")
            for half in range(2):
                hs = slice(512 * half, 512 * (half + 1))
                nc.tensor.matmul(
                    prbc[:, hs], ones1r, Rr[0:1, hs], start=True, stop=True
                )
            rbc = sbx.tile([128, N], f32, tag="rbc")
            nc.vector.tensor_copy(rbc, prbc)

            # ---- QKV ----
            qT = [sbt.tile([128, N], bf16, tag=f"qT{j}", name=f"qT{j}")
                  for j in range(4)]
            kT = [sbt.tile([128, N], bf16, tag=f"kT{j}", name=f"kT{j}")
                  for j in range(4)]
            v_sb = [sbt.tile([128, 512], bf16, tag=f"v{m}", name=f"v{m}")
                    for m in range(8)]

            def emit_qk(j, w_tiles, cb, dest):
                pg = psA.tile([128, N], f32, tag="A", name="pg")
                cs = slice(128 * j, 128 * (j + 1))
                for c in range(3):
                    for half in range(2):
                        hs = slice(512 * half, 512 * (half + 1))
                        nc.tensor.matmul(
                            pg[:, hs], w_tiles[c][:, cs], xb_sb[c][:, hs],
                            start=(c == 0), stop=False,
                        )
                for half in range(2):
                    hs = slice(512 * half, 512 * (half + 1))
                    nc.tensor.matmul(
                        pg[:, hs], cb[:, cs], qv2[:, hs],
                        start=False, stop=True,
                    )
                nc.vector.tensor_tensor(dest, pg, rbc, MULT)

            def emit_v(m):
                pgv = psA.tile([128, 512], f32, tag="A", name="pgv")
                ms = slice(128 * m, 128 * (m + 1))
                for c in range(3):
                    nc.tensor.matmul(
                        pgv, xb_sb[c][:, ms], wv_sb[c],
                        start=(c == 0), stop=False,
                    )
                nc.tensor.matmul(
                    pgv, qv2[:, ms], cbv_sb, start=False, stop=True
                )
                nc.vector.tensor_scalar(
                    out=v_sb[m], in0=pgv, scalar1=rcol[:, m:m + 1],
                    scalar2=None, op0=MULT,
                )
                nc.vector.memset(v_sb[m][:, slice(0, 512, 64)], 1.0)

            attnT = [sbt.tile([128, N], bf16, tag=f"at{j}", name=f"at{j}")
                     for j in range(4)]

            emit_qk(0, wq_sb, cbq_sb, qT[0])
            emit_qk(0, wk_sb, cbk_sb, kT[0])
            emit_v(0)
            emit_v(1)

            # ---- attention, one head pair at a time ----
            for j in range(PAIRS):
                pav = psC.tile([128, N], f32, tag="C", name="pav")
                for m in range(8):
                    # prefetch upcoming v tiles / next pair's q,k
                    if j == 0 and m + 2 < 8:
                        emit_v(m + 2)
                    if m == 3 and j + 1 < PAIRS:
                        emit_qk(j + 1, wq_sb, cbq_sb, qT[j + 1])
                    if m == 6 and j + 1 < PAIRS:
                        emit_qk(j + 1, wk_sb, cbk_sb, kT[j + 1])
                    msl = slice(128 * m, 128 * (m + 1))
                    s_t = [psA.tile([128, N], f32, tag="A", name="s")
                           for _ in range(2)]
                    # group same-lhsT matmuls so dup LDWEIGHTS dedup
                    for half in range(2):
                        nc.tensor.matmul(
                            s_t[half][:, 0:512],
                            kT[j][0:64, msl],
                            qT[j][0:64, slice(512 * half, 512 * (half + 1))],
                            start=True, stop=True,
                        )
                    for half in range(2):
                        nc.tensor.matmul(
                            s_t[half][:, 512:1024],
                            kT[j][64:128, msl],
                            qT[j][64:128, slice(512 * half, 512 * (half + 1))],
                            start=True, stop=True, tile_position=(64, 0),
                        )
                    u_tiles = []
                    for half in range(2):
                        u_t = sbu.tile([128, N], bf16, tag="U", name="u")
                        nc.scalar.activation(u_t, s_t[half], AF.Exp, scale=SCALE)
                        u_tiles.append(u_t)
                    # AV: group by head so consecutive matmuls share lhsT
                    for half in range(2):
                        hs = slice(512 * half, 512 * (half + 1))
                        nc.tensor.matmul(
                            pav[0:64, hs],
                            v_sb[m][:, 128 * j:128 * j + 64],
                            u_tiles[half][:, 0:512],
                            start=(m == 0), stop=(m == 7),
                            tile_position=(0, 0),
                        )
                    for half in range(2):
                        hs = slice(512 * half, 512 * (half + 1))
                        nc.tensor.matmul(
                            pav[64:128, hs],
                            v_sb[m][:, 128 * j + 64:128 * (j + 1)],
                            u_tiles[half][:, 512:1024],
                            start=(m == 0), stop=(m == 7),
                            tile_position=(0, 64),
                        )
                # denominators live at rows 0 (head A) and 64 (head B)
                av_sb = sbs.tile([128, N], f32, tag="av_sb")
                if j < PAIRS - 1:
                    dA = sbr.tile([1, N], f32, tag="dA")
                    dB = sbr.tile([1, N], f32, tag="dB")
                    nc.vector.tensor_copy(dA, pav[0:1, :])
                    nc.vector.tensor_copy(dB, pav[64:65, :])
                    nc.vector.tensor_copy(av_sb, pav)
                    nc.sync.dma_start(out=dscr_d[j][0:1, :], in_=dA)
                    nc.sync.dma_start(out=dscr_d[j][1:2, :], in_=dB)
                    rbraw = sbs.tile([128, N], f32, tag="rbraw")
                    srcA = bass.AP(tensor=dscr_d[j], offset=0,
                                   ap=[[0, 64], [1, N]])
                    srcB = bass.AP(tensor=dscr_d[j], offset=N,
                                   ap=[[0, 64], [1, N]])
                    nc.gpsimd.dma_start(out=rbraw[0:64, :], in_=srcA)
                    nc.gpsimd.dma_start(out=rbraw[64:128, :], in_=srcB)
                    rb_sb = sbs.tile([128, N], f32, tag="rb")
                    nc.vector.reciprocal_approx_fast(out=rb_sb, in_=rbraw)
                    nc.vector.tensor_tensor(attnT[j], av_sb, rb_sb, MULT)
                else:
                    # last pair: keep the reciprocal chain on-chip (latency)
                    dAr = sbr.tile([1, N], f32r, tag="dAr")
                    dBr = sbr.tile([1, N], f32r, tag="dBr")
                    with nc.allow_low_precision(reason="f32r denom copy"):
                        nc.vector.tensor_copy(dAr, pav[0:1, :])
                        nc.vector.tensor_copy(dBr, pav[64:65, :])
                    nc.vector.tensor_copy(av_sb, pav)
                    prb = psA.tile([128, N], f32, tag="A", name="prb")
                    for half in range(2):
                        hs = slice(512 * half, 512 * (half + 1))
                        nc.tensor.matmul(
                            prb[:, hs], maskA, dAr[0:1, hs],
                            start=True, stop=False,
                        )
                        nc.tensor.matmul(
                            prb[:, hs], maskB, dBr[0:1, hs],
                            start=False, stop=True,
                        )
                    rb_sb = sbs.tile([128, N], f32, tag="rb")
                    nc.vector.reciprocal_approx_fast(out=rb_sb, in_=prb)
                    nc.vector.tensor_tensor(attnT[j], av_sb, rb_sb, MULT)

            # ---- projection + residual ----
            for p in range(3):
                pp = psA.tile([128, N], f32, tag="A", name="pp")
                cs = slice(128 * p, 128 * (p + 1))
                for r in range(4):
                    for half in range(2):
                        hs = slice(512 * half, 512 * (half + 1))
                        nc.tensor.matmul(
                            pp[:, hs], wp_sb[r][:, cs], attnT[r][:, hs],
                            start=(r == 0), stop=(r == 3),
                        )
                o_sb = sbx.tile([128, N], f32, tag=f"o{p}", name=f"o{p}")
                nc.vector.tensor_tensor(o_sb, pp, x_sb[p], ADD)
                nc.sync.dma_start(out=out_d[cs, :], in_=o_sb)

    nc.compile()
    _dedup_ldweights(nc)
    return nc


def _dedup_ldweights(nc):
    """Drop LDWEIGHTS that reload the exact weights already resident in the
    same PE-array region (tile_position/tile_size), with no intervening
    overlapping load. Only drops instructions carrying no waits/updates."""

    def rect(tp, ts):
        return (tp[0], tp[0] + ts[0], tp[1], tp[1] + ts[1])

    def overlaps(r1, r2):
        return r1[0] < r2[1] and r2[0] < r1[1] and r1[2] < r2[3] and r2[2] < r1[3]

    for f in nc.m.functions:
        for blk in f.blocks:
            region = {}
            out = []
            for inst in blk.instructions:
                t = type(inst).__name__
                if t == "InstLdweights":
                    tp = tuple(inst.tile_position)
                    ts = tuple(inst.tile_size)
                    key = rect(tp, ts)
                    sig = (str(inst.ins[0]), str(inst.perf_mode))
                    if (
                        region.get(key) == sig
                        and not inst.has_wait()
                        and not inst.has_update()
                    ):
                        continue
                    for k2 in list(region):
                        if overlaps(key, k2):
                            del region[k2]
                    region[key] = sig
                elif t == "InstMatmult":
                    if inst.ldweights is not False:
                        region.clear()
                out.append(inst)
            blk.instructions = out


def _host_prep(w_qkv, w_proj, ln_w, ln_b):
    """Build padded/transposed weight layouts shared by all cores."""
    f32 = np.float32
    bf = ml_dtypes.bfloat16
    Wp = (w_qkv * ln_w[None, :]).astype(f32)          # [3C, C]
    bias = (w_qkv @ ln_b).astype(f32)                 # [3C]
    colsum = Wp.sum(axis=1).astype(f32)               # [3C]

    wq = np.zeros((C, 512), f32)
    wk = np.zeros((C, 512), f32)
    wv = np.zeros((C, 512), f32)
    cbq = np.zeros((2, 512), f32)
    cbk = np.zeros((2, 512), f32)
    cbv = np.zeros((2, 512), f32)
    for h in range(NH):
        qsl = slice(64 * h, 64 * h + D)
        wq[:, qsl] = Wp[48 * h:48 * h + D, :].T
        wk[:, qsl] = Wp[C + 48 * h:C + 48 * h + D, :].T
        cbq[0, qsl] = colsum[48 * h:48 * h + D]
        cbq[1, qsl] = bias[48 * h:48 * h + D]
        cbk[0, qsl] = colsum[C + 48 * h:C + 48 * h + D]
        cbk[1, qsl] = bias[C + 48 * h:C + 48 * h + D]
        j, odd = divmod(h, 2)
        base = 128 * j + 64 * odd + 1
        vsl = slice(base, base + D)
        wv[:, vsl] = Wp[2 * C + 48 * h:2 * C + 48 * h + D, :].T
        cbv[0, vsl] = colsum[2 * C + 48 * h:2 * C + 48 * h + D]
        cbv[1, vsl] = bias[2 * C + 48 * h:2 * C + 48 * h + D]

    wp = np.zeros((512, C), f32)
    for h in range(NH):
        j, odd = divmod(h, 2)
        base = 128 * j + 64 * odd + 1
        wp[base:base + D, :] = w_proj[:, 48 * h:48 * h + D].T

    consts = {
        "wq": wq.astype(bf), "wk": wk.astype(bf), "wv": wv.astype(bf),
        "wp": wp.astype(bf),
        "cbq": cbq.astype(bf), "cbk": cbk.astype(bf), "cbv": cbv.astype(bf),
        "ones128": np.ones((128, 1), bf),
        "ones1r": np.ones((1, 128), f32),
        "maskA": np.concatenate(
            [np.ones((1, 64), f32), np.zeros((1, 64), f32)], axis=1
        ),
        "maskB": np.concatenate(
            [np.zeros((1, 64), f32), np.ones((1, 64), f32)], axis=1
        ),
    }
    return consts


def kernel(x, w_qkv, w_proj, ln_w, ln_b):
    from concourse.bass_utils import run_bass_kernel_spmd

    x = np.asarray(x, dtype=np.float32)
    w_qkv = np.asarray(w_qkv, dtype=np.float32)
    w_proj = np.asarray(w_proj, dtype=np.float32)
    ln_w = np.asarray(ln_w, dtype=np.float32)
    ln_b = np.asarray(ln_b, dtype=np.float32)

    if "nc" not in _CACHE:
        _CACHE["nc"] = _build()
    nc = _CACHE["nc"]

    consts = _host_prep(w_qkv, w_proj, ln_w, ln_b)
    in_maps = []
    for i in range(B):
        xi = np.ascontiguousarray(x[i].reshape(C, N))
        m = {"x": xi, "xb": xi.astype(ml_dtypes.bfloat16)}
        m.update(consts)
        in_maps.append(m)

    res = run_bass_kernel_spmd(nc, in_maps, list(range(B)))
    out = np.stack([res.results[i]["out"].reshape(C, 32, 32) for i in range(B)])
    return out.astype(np.float32)
